# revision 11
# baseline (speedup 1.0000x reference)
"""CompGCN message-passing kernel for 8 Trainium2 NeuronCores.

Computes (from reference):
    ho = segment_sum(node[src] - edge, dst)   # [N, D]
    hi = segment_sum(node[dst] - edge, src)   # [N, D]
    h  = ho @ W_O.T + b_O + hi @ W_I.T + b_I  # [N, D]
    he = edge @ W_rel.T + b_rel               # [E, D]

Strategy (no collectives):
  - Two passes over edges, one sorted by dst (for ho) and one by src (for hi).
    Cores own disjoint 128-node blocks of the scatter index, so partial
    aggregates never need an all-reduce.
  - Node rows are gathered from HBM with the GPSIMD dma_gather instruction
    (512-byte descriptors at full bus rate).  Its indices are int16, so the
    node table is split at row 32768 and each 128-node destination block's
    edges are grouped into an "A" segment (gather row < 32768) and a "B"
    segment (the rest, gathered from a base-offset table).  The f32 rows are
    moved bitcast as bf16 pairs (dma_gather is a byte mover).
  - The segment sum within a block is a matmul with a one-hot selection
    matrix S built on DVE via is_equal against an iota matrix; PSUM
    accumulates the block over all of its edge tiles, directly in transposed
    ([D, node]) layout so the output projection is a stationary-weight matmul.
  - Edges arrive pre-negated, m = gather + (-edge) on DVE, one accumulating
    matmul per 128-edge tile.
  - he is computed from a host-transposed edge view in [D, E] layout with
    W_rel.T stationary, bias added by the scalar engine during PSUM eviction.
"""

import numpy as np

import concourse.mybir as mybir
from concourse import bacc
from concourse.bass_utils import run_bass_kernel_spmd
from concourse.tile import TileContext

P = 128
N_CORES = 8
HALF = 32768  # dma_gather int16 index limit
MAX_BATCH = 8  # tiles per dma_gather (ucode limit: num_idxs <= 1024)
HE_TILE = 512

# test.py hooks (harness never touches these)
TRACE = False
TRACE_KWARGS = {}
LAST_RESULTS = None

_cache = {}


def _plan_pass(scatter_idx, gather_idx, n_nodes, nb_per_core):
    """Bucket edges by 128-node block of scatter_idx per core; within a block
    order by gather-table half (A: gather < HALF, B: rest), each segment
    padded to whole 128-edge tiles.

    Returns dict with:
      TA, TB: [nb] tiles per block segment (shared across cores; TA >= 1)
      batches: list of (tile0, ntiles, is_b) covering all TT tiles in order
      blocks: per tile -> block id;  first/last tile per block
      eid_slots: [n_cores, TT*128] int64 edge id per slot (0 for pads)
      idx16: [n_cores, 128, TT*8] int16 packed gather indices
      sloc: [n_cores, 128, TT] float32 local scatter index (-1 pads)
    """
    nodes_per_core = nb_per_core * P
    core_of_edge = scatter_idx // nodes_per_core
    cntA = np.zeros((N_CORES, nb_per_core), dtype=np.int64)
    cntB = np.zeros((N_CORES, nb_per_core), dtype=np.int64)
    per_core = []
    for c in range(N_CORES):
        eids = np.nonzero(core_of_edge == c)[0]
        loc = scatter_idx[eids] - c * nodes_per_core
        blk = loc // P
        isb = (gather_idx[eids] >= HALF).astype(np.int64)
        order = np.lexsort((isb, blk))
        eids, loc, blk, isb = eids[order], loc[order], blk[order], isb[order]
        cntA[c] = np.bincount(blk[isb == 0], minlength=nb_per_core)
        cntB[c] = np.bincount(blk[isb == 1], minlength=nb_per_core)
        per_core.append((eids, loc, blk, isb))
    TA = np.maximum(1, -(-cntA.max(axis=0) // P))
    TB = -(-cntB.max(axis=0) // P)
    TT = int(TA.sum() + TB.sum())

    # tile stream: per block, TA[j] A-tiles then TB[j] B-tiles
    offA = np.zeros(nb_per_core, dtype=np.int64)
    offB = np.zeros(nb_per_core, dtype=np.int64)
    blocks = np.zeros(TT, dtype=np.int64)
    batches = []
    t = 0
    for j in range(nb_per_core):
        offA[j] = t
        for s0 in range(0, int(TA[j]), MAX_BATCH):
            batches.append((t + s0, min(MAX_BATCH, int(TA[j]) - s0), False))
        blocks[t : t + int(TA[j])] = j
        t += int(TA[j])
        offB[j] = t
        for s0 in range(0, int(TB[j]), MAX_BATCH):
            batches.append((t + s0, min(MAX_BATCH, int(TB[j]) - s0), True))
        blocks[t : t + int(TB[j])] = j
        t += int(TB[j])
    assert t == TT
    first_tile = np.zeros(nb_per_core, dtype=np.int64)
    last_tile = np.zeros(nb_per_core, dtype=np.int64)
    for j in range(nb_per_core):
        first_tile[j] = offA[j]
        last_tile[j] = offB[j] + int(TB[j]) - 1 if TB[j] > 0 else offA[j] + int(TA[j]) - 1

    eid_slots = np.zeros((N_CORES, TT * P), dtype=np.int64)
    idx_slots = np.zeros((N_CORES, TT * P), dtype=np.int16)
    sloc = np.full((N_CORES, P, TT), -1.0, dtype=np.float32)
    for c in range(N_CORES):
        eids, loc, blk, isb = per_core[c]
        cumA = np.concatenate([[0], np.cumsum(cntA[c])])
        cumB = np.concatenate([[0], np.cumsum(cntB[c])])
        rank = np.where(
            isb == 0,
            np.arange(len(eids)) - (cumA[blk] + cumB[blk]),
            np.arange(len(eids)) - (cumA[blk] + cumB[blk] + cntA[c][blk]),
        )
        slot = np.where(isb == 0, offA[blk], offB[blk]) * P + rank
        eid_slots[c, slot] = eids
        g = gather_idx[eids]
        idx_slots[c, slot] = np.where(isb == 0, g, g - HALF).astype(np.int16)
        sloc[c, slot % P, slot // P] = (loc - blk * P).astype(np.float32)
    # pack indices: idx j of a batch lives at [16g + j%16, j//16]; batches are
    # 128-slot aligned so a global [16, TT*8] layout tiled to 128 partitions works.
    idx16 = np.ascontiguousarray(
        np.tile(idx_slots.reshape(N_CORES, TT * 8, 16).transpose(0, 2, 1), (1, 8, 1))
    )
    return dict(
        TA=TA, TB=TB, TT=TT, batches=batches, blocks=blocks,
        first_tile=first_tile, last_tile=last_tile,
        eid_slots=eid_slots, idx16=idx16, sloc=sloc,
    )


def _build_nc(n_nodes, nb_per_core, e_core, plan_d, plan_s, D):
    f32 = mybir.dt.float32
    i16 = mybir.dt.int16
    bf16 = mybir.dt.bfloat16
    nA = min(n_nodes, HALF)
    nB = n_nodes - nA
    TTd, TTs = plan_d["TT"], plan_s["TT"]

    nc = bacc.Bacc("TRN2", target_bir_lowering=False)

    nodeA = nc.declare_dram_parameter("nodeA", [nA, 2 * D], bf16, isOutput=False)
    nodeB = (
        nc.declare_dram_parameter("nodeB", [nB, 2 * D], bf16, isOutput=False)
        if nB > 0
        else None
    )
    eneg_d = nc.declare_dram_parameter("eneg_d", [TTd * P, D], f32, isOutput=False)
    idx_d = nc.declare_dram_parameter("idx_d", [P, TTd * 8], i16, isOutput=False)
    sloc_d = nc.declare_dram_parameter("sloc_d", [P, TTd], f32, isOutput=False)
    eneg_s = nc.declare_dram_parameter("eneg_s", [TTs * P, D], f32, isOutput=False)
    idx_s = nc.declare_dram_parameter("idx_s", [P, TTs * 8], i16, isOutput=False)
    sloc_s = nc.declare_dram_parameter("sloc_s", [P, TTs], f32, isOutput=False)
    ET = nc.declare_dram_parameter("ET", [P, e_core], f32, isOutput=False)
    Jr = nc.declare_dram_parameter("Jrep", [P, MAX_BATCH * P], f32, isOutput=False)
    WoT = nc.declare_dram_parameter("WoT", [D, D], f32, isOutput=False)
    WiT = nc.declare_dram_parameter("WiT", [D, D], f32, isOutput=False)
    WrT = nc.declare_dram_parameter("WrT", [D, D], f32, isOutput=False)
    bOI = nc.declare_dram_parameter("bOI", [D, 1], f32, isOutput=False)
    brel = nc.declare_dram_parameter("brel", [D, 1], f32, isOutput=False)
    hT = nc.declare_dram_parameter("hT", [D, nb_per_core * P], f32, isOutput=True)
    heT = nc.declare_dram_parameter("heT", [D, e_core], f32, isOutput=True)

    is_eq = mybir.AluOpType.is_equal
    ident = mybir.ActivationFunctionType.Identity

    with TileContext(nc) as tc:
        with (
            tc.tile_pool(name="const", bufs=1) as cpool,
            tc.tile_pool(name="agg", bufs=1) as apool,
            tc.tile_pool(name="slab", bufs=2) as slab_pool,
            tc.tile_pool(name="stage", bufs=4) as st_pool,
            tc.tile_pool(name="psum", bufs=2, space="PSUM") as psum_pool,
            tc.tile_pool(name="hepsum", bufs=2, space="PSUM") as he_psum,
        ):
            J_sb = cpool.tile([P, MAX_BATCH * P], f32, tag="J")
            nc.sync.dma_start(out=J_sb[:], in_=Jr[:])
            WoT_sb = cpool.tile([D, D], f32, tag="WoT")
            nc.sync.dma_start(out=WoT_sb[:], in_=WoT[:])
            WiT_sb = cpool.tile([D, D], f32, tag="WiT")
            nc.sync.dma_start(out=WiT_sb[:], in_=WiT[:])
            WrT_sb = cpool.tile([D, D], f32, tag="WrT")
            nc.sync.dma_start(out=WrT_sb[:], in_=WrT[:])
            bOI_sb = cpool.tile([D, 1], f32, tag="bOI")
            nc.sync.dma_start(out=bOI_sb[:], in_=bOI[:])
            brel_sb = cpool.tile([D, 1], f32, tag="brel")
            nc.sync.dma_start(out=brel_sb[:], in_=brel[:])

            hoT_sb = apool.tile([D, nb_per_core * P], f32, tag="hoT")
            hiT_sb = apool.tile([D, nb_per_core * P], f32, tag="hiT")

            def emit_pass(eneg, idx_p, sloc_p, plan, agg_sb, tag):
                TT = plan["TT"]
                idx_sb = cpool.tile([P, TT * 8], i16, tag=f"idx{tag}")
                nc.sync.dma_start(out=idx_sb[:], in_=idx_p[:])
                sloc_sb = cpool.tile([P, TT], f32, tag=f"sloc{tag}")
                nc.sync.dma_start(out=sloc_sb[:], in_=sloc_p[:])

                blocks = plan["blocks"]
                first_tile = plan["first_tile"]
                last_tile = plan["last_tile"]
                ps = None
                for t0, nt, is_b in plan["batches"]:
                    ni = nt * P
                    esl = slab_pool.tile([P, MAX_BATCH * D], f32, tag="esl")
                    nc.sync.dma_start(
                        out=esl[:, : nt * D].rearrange("p (a f) -> p a f", f=D),
                        in_=eneg[t0 * P : (t0 + nt) * P, :].rearrange(
                            "(a p) f -> p a f", p=P
                        ),
                    )
                    gbuf = slab_pool.tile([P, MAX_BATCH * 2 * D], bf16, tag="gbuf")
                    nc.gpsimd.dma_gather(
                        gbuf[:, : nt * 2 * D].rearrange("p (a f) -> p a f", f=2 * D),
                        (nodeB if is_b else nodeA)[:],
                        idx_sb[:, t0 * 8 : (t0 + nt) * 8],
                        ni,
                        ni,
                        2 * D,
                    )
                    mb = slab_pool.tile([P, MAX_BATCH * D], f32, tag="mb")
                    nc.vector.tensor_add(
                        out=mb[:, : nt * D],
                        in0=gbuf[:, : nt * 2 * D].bitcast(f32),
                        in1=esl[:, : nt * D],
                    )
                    Sb = slab_pool.tile([P, MAX_BATCH * P], f32, tag="S")
                    nc.vector.tensor_tensor(
                        out=Sb[:, : nt * P].rearrange("p (t j) -> p t j", j=P),
                        in0=sloc_sb[:, t0 : t0 + nt].to_broadcast([P, nt, P]),
                        in1=J_sb[:, : nt * P].rearrange("p (t j) -> p t j", j=P),
                        op=is_eq,
                    )
                    for k in range(nt):
                        t = t0 + k
                        j = int(blocks[t])
                        if t == int(first_tile[j]):
                            ps = psum_pool.tile([P, P], f32, tag="ps")
                        nc.tensor.matmul(
                            out=ps[:],
                            lhsT=mb[:, k * D : (k + 1) * D],
                            rhs=Sb[:, k * P : (k + 1) * P],
                            start=(t == int(first_tile[j])),
                            stop=(t == int(last_tile[j])),
                        )
                        if t == int(last_tile[j]):
                            nc.scalar.copy(
                                out=agg_sb[:, j * P : (j + 1) * P], in_=ps[:]
                            )

            emit_pass(eneg_d, idx_d, sloc_d, plan_d, hoT_sb, "d")
            emit_pass(eneg_s, idx_s, sloc_s, plan_s, hiT_sb, "s")

            # projection: hT = W_O @ hoT + W_I @ hiT + (b_O + b_I)
            for j in range(nb_per_core):
                pp = psum_pool.tile([P, P], f32, tag="ps")
                nc.tensor.matmul(
                    out=pp[:],
                    lhsT=WoT_sb[:],
                    rhs=hoT_sb[:, j * P : (j + 1) * P],
                    start=True,
                    stop=False,
                )
                nc.tensor.matmul(
                    out=pp[:],
                    lhsT=WiT_sb[:],
                    rhs=hiT_sb[:, j * P : (j + 1) * P],
                    start=False,
                    stop=True,
                )
                hstage = st_pool.tile([P, P], f32, tag="hstage")
                nc.scalar.activation(
                    out=hstage[:], in_=pp[:], func=ident, bias=bOI_sb[:, 0:1], scale=1.0
                )
                nc.sync.dma_start(out=hT[:, j * P : (j + 1) * P], in_=hstage[:])

            # he: heT = W_rel @ ET + b_rel
            for s0 in range(0, e_core, HE_TILE):
                w = min(HE_TILE, e_core - s0)
                et = st_pool.tile([P, HE_TILE], f32, tag="et")
                nc.sync.dma_start(out=et[:, :w], in_=ET[:, s0 : s0 + w])
                php = he_psum.tile([P, HE_TILE], f32, tag="hep")
                nc.tensor.matmul(
                    out=php[:, :w], lhsT=WrT_sb[:], rhs=et[:, :w], start=True, stop=True
                )
                hes = st_pool.tile([P, HE_TILE], f32, tag="hes")
                nc.scalar.activation(
                    out=hes[:, :w],
                    in_=php[:, :w],
                    func=ident,
                    bias=brel_sb[:, 0:1],
                    scale=1.0,
                )
                nc.sync.dma_start(out=heT[:, s0 : s0 + w], in_=hes[:, :w])

    nc.finalize()
    return nc


def kernel(node_embs, edge_embs, src, dst, W_O, b_O, W_I, b_I, W_rel, b_rel):
    global LAST_RESULTS
    import ml_dtypes

    node_embs = np.ascontiguousarray(np.asarray(node_embs, dtype=np.float32))
    edge_embs = np.ascontiguousarray(np.asarray(edge_embs, dtype=np.float32))
    src = np.asarray(src).astype(np.int64)
    dst = np.asarray(dst).astype(np.int64)
    W_O = np.asarray(W_O, dtype=np.float32)
    b_O = np.asarray(b_O, dtype=np.float32)
    W_I = np.asarray(W_I, dtype=np.float32)
    b_I = np.asarray(b_I, dtype=np.float32)
    W_rel = np.asarray(W_rel, dtype=np.float32)
    b_rel = np.asarray(b_rel, dtype=np.float32)

    n_nodes, D = node_embs.shape
    E = edge_embs.shape[0]
    assert E % N_CORES == 0, E
    e_core = E // N_CORES
    nb_per_core = -(-n_nodes // (N_CORES * P))  # ceil

    plan_d = _plan_pass(dst, src, n_nodes, nb_per_core)
    plan_s = _plan_pass(src, dst, n_nodes, nb_per_core)

    edge_neg = -edge_embs
    eneg_d = edge_neg[plan_d["eid_slots"].reshape(-1)].reshape(
        N_CORES, plan_d["TT"] * P, D
    )
    eneg_s = edge_neg[plan_s["eid_slots"].reshape(-1)].reshape(
        N_CORES, plan_s["TT"] * P, D
    )
    ETfull = np.ascontiguousarray(edge_embs.T)  # [D, E]

    nA = min(n_nodes, HALF)
    nodeA_v = node_embs[:nA].view(ml_dtypes.bfloat16)
    nodeB_v = (
        np.ascontiguousarray(node_embs[nA:]).view(ml_dtypes.bfloat16)
        if n_nodes > nA
        else None
    )

    Jrep = np.tile(np.arange(P, dtype=np.float32), (P, MAX_BATCH))
    WoT = np.ascontiguousarray(W_O.T)
    WiT = np.ascontiguousarray(W_I.T)
    WrT = np.ascontiguousarray(W_rel.T)
    bOI = np.ascontiguousarray((b_O + b_I).reshape(D, 1))
    brel = np.ascontiguousarray(b_rel.reshape(D, 1))

    key = (
        n_nodes, nb_per_core, e_core, D,
        tuple(plan_d["TA"]), tuple(plan_d["TB"]),
        tuple(plan_s["TA"]), tuple(plan_s["TB"]),
    )
    if key not in _cache:
        _cache[key] = _build_nc(n_nodes, nb_per_core, e_core, plan_d, plan_s, D)
    nc = _cache[key]

    in_maps = []
    for c in range(N_CORES):
        m = {
            "nodeA": nodeA_v,
            "eneg_d": np.ascontiguousarray(eneg_d[c]),
            "idx_d": np.ascontiguousarray(plan_d["idx16"][c]),
            "sloc_d": np.ascontiguousarray(plan_d["sloc"][c]),
            "eneg_s": np.ascontiguousarray(eneg_s[c]),
            "idx_s": np.ascontiguousarray(plan_s["idx16"][c]),
            "sloc_s": np.ascontiguousarray(plan_s["sloc"][c]),
            "ET": np.ascontiguousarray(ETfull[:, c * e_core : (c + 1) * e_core]),
            "Jrep": Jrep,
            "WoT": WoT,
            "WiT": WiT,
            "WrT": WrT,
            "bOI": bOI,
            "brel": brel,
        }
        if nodeB_v is not None:
            m["nodeB"] = nodeB_v
        in_maps.append(m)

    res = run_bass_kernel_spmd(
        nc, in_maps, core_ids=list(range(N_CORES)), trace=TRACE, **TRACE_KWARGS
    )
    LAST_RESULTS = res

    h = np.concatenate([r["hT"] for r in res.results], axis=1).T[:n_nodes]
    he = np.concatenate([r["heT"] for r in res.results], axis=1).T
    return (np.ascontiguousarray(h), np.ascontiguousarray(he))
